# revision 56
# baseline (speedup 1.0000x reference)
"""Trainium2 Bass kernel for nn_DenseAttentionOneHead (B=2, L=4096, H=1024).

Reference math:
    h   = hidden * cos + rotate_half(hidden) * sin      (RoPE)
    q   = h @ W_q.T
    out = (q @ h^T) @ h                                 (no softmax)

With no softmax the L x L score matrix factorizes away and W_q folds in:
    out[b] = h[b] @ M[b],  M[b] = W_q^T G[b],  G[b] = h[b]^T h[b]  (H x H)

Sharding (8 NeuronCores, ZERO collectives): cores are column-parallel
inside each batch. Core (b, j) computes a 256-column slice of G, M and y
for batch b with the FULL L=4096 contraction, reading all of h[b]. The
cost-model collectives (15us fixed + bytes/40GBps each) priced the old
row-parallel ReduceScatter+AllGather exchange at ~95us of serial time;
re-reading h (8MB bf16, ~23us of DMA overlapped under compute) is far
cheaper.

All 8 cores run one SPMD module: the host rolls the RoPE *pair* axis by
128*j so every core's slice lands at permuted columns {0:128, 512:640},
and permutes W_q on both axes to match (contractions are order-agnostic).
The host un-permutes the output columns when scattering back.

Per-core phases (~92.6us simulated; phase 1 paced by the 1456ns/tile DMA):
  1. Stream 32 l-tiles of packed [h|cos|sin] (one 4KB-desc DMA each).
     RoPE as two 1024-wide DVE muls — [h1|h2]*[c|s] and a negative-stride
     swapped view [h2|h1]*[c|s] — plus a sub (DVE, Pool every 4th tile)
     and an add (Pool), keeping DVE ~1431ns and Pool ~1389ns per tile,
     both under the DMA pace. G[:, slice] accumulates into 4 persistent
     PSUM banks (2 row-blocks per bank; only the bank's first group may
     use start=True since start zeroes the whole bank). Each tile is also
     PE-transposed (bf16, 1 cyc/row) into hT with ACT draining the
     transpose banks. At the stream tail the DMA is done, so the last
     tiles' RoPE tail ops collapse onto DVE, the final hT drains are
     deferred (y only reads them ~30us later), and the G drain goes out
     on DVE — shortening the phase-1-to-M handoff by ~2us.
  2. M[:, slice] = W_q^T G[:, slice]: 64 matmuls, W arriving right
     behind the h stream on the same DMA ring. A few no-reader PE filler
     matmuls bridge the drain/DMA waits so the p-state never drops.
  3. yT[slice, :] = M[:, slice]^T hT: transposed orientation gives
     512-row moving operands (128 matmuls); PSUM drained via DVE/ACT
     casts to bf16 and DMA'd out; the host re-transposes when scattering.
"""

import os

import numpy as np

import jax

try:
    _cache_dir = os.path.join(os.path.expanduser("~"), ".cache", "bass_kernel_jax")
    os.makedirs(_cache_dir, exist_ok=True)
    jax.config.update("jax_compilation_cache_dir", _cache_dir)
    jax.config.update("jax_persistent_cache_min_compile_time_secs", 1.0)
except Exception:
    pass

import ml_dtypes

import concourse.bacc as bacc
import concourse.bass as bass
import concourse.mybir as mybir
import concourse.tile as tile
from concourse import masks
from concourse.bass_utils import run_bass_kernel_spmd

F32 = mybir.dt.float32
BF16 = mybir.dt.bfloat16
BF16_NP = ml_dtypes.bfloat16

B, L, H = 2, 4096, 1024
HH = H // 2          # 512 RoPE pairs
NT = L // 128        # 32 l-tiles
NCS = 256            # columns of G/M/y per core
PAIRS = 128          # RoPE pairs per core slice

BYP = mybir.AluOpType.bypass
MUL = mybir.AluOpType.mult
ADD = mybir.AluOpType.add
SUB = mybir.AluOpType.subtract


def _emit_once(nc, tc, hcs_d, wq_d, y_d):
    # hcs rows are l; cols = [h_perm (1024) | cos_perm (512) | sin_perm (512)]
    hcs_ap = hcs_d.ap().rearrange("(t p) c -> p t c", p=128)  # [128, 32, 2048]
    wq_ap = wq_d.ap().rearrange("(t p) c -> p t c", p=128)    # [128, 8, 1024]
    y_ap = y_d.ap()                                           # yT [256, 4096]

    with (
        tc.tile_pool(name="persist", bufs=1) as persist,
        tc.tile_pool(name="stream", bufs=1) as stream,
        tc.tile_pool(name="pacc", bufs=1, space="PSUM") as pacc,
    ):
        hT = persist.tile([128, 8, L], BF16, name="hT")        # h^T, 64KB/part
        wq_sb = persist.tile([128, 8, H], BF16, name="wq_sb")  # W_q, 16KB/part
        gsl = persist.tile([128, 8, NCS], BF16, name="gsl")    # G[:, slice]
        mq = persist.tile([128, 8, NCS], BF16, name="mq")      # M[:, slice]
        dum = persist.tile([128, 512], BF16, name="dum")       # filler operand
        nc.vector.memset(dum[:], 0.0)

        identf = stream.tile([128, 128], F32, name="identf")
        masks.make_identity(nc, identf[:])
        identb = stream.tile([128, 128], BF16, name="identb")
        nc.vector.tensor_copy(identb[:], identf[:])

        fill = pacc.tile([128, 512], F32, name="fill", tag="fill", bufs=1)

        def fillers(n, rows=512):
            # no-reader self-matmuls keep the PE p-state ramped across
            # short DMA/copy waits (post-idle matmuls cost up to 2x)
            for _ in range(n):
                nc.tensor.matmul(
                    fill[:, 0:rows], dum[:, 0:128], dum[:, 0:rows],
                    start=True, stop=True, skip_group_check=True,
                )

        # G[:, slice] accumulators: 4 banks, 2 row-blocks each, packed
        # [ob_even: lo|hi, ob_odd: lo|hi] = [0:128|128:256, 256:384|384:512]
        gacc = [
            pacc.tile([128, 512], F32, name=f"gacc{i}", tag="acc", bufs=4)
            for i in range(4)
        ]

        deferred_ht = []

        # ---- phase 1: stream h|c|s, RoPE, G accumulation, transposes ----
        for t in range(NT):
            hc = stream.tile([128, 2048], BF16, name="hc", tag="ld", bufs=6)
            nc.sync.dma_start(hc[:], hcs_ap[:, t, :])
            hr = stream.tile([128, H], BF16, name="hr", tag="hr", bufs=6)
            mA = stream.tile([128, 2, HH], BF16, name="mA", tag="mA", bufs=4)
            mB = stream.tile([128, 2, HH], BF16, name="mB", tag="mB", bufs=4)
            # RoPE via two 1024-wide DVE muls (amortizes the access-latency
            # bubble): mA = [h1|h2]*[c|s] = [m1|m2]; mB = [h2|h1]*[c|s]
            # (negative-stride swapped view) = [m3|m4]. Then
            # hr_lo = m1 - m2 (DVE; Pool every 4th tile), hr_hi = m3 + m4
            # (Pool) — DVE/Pool both stay under the 1456ns/tile DMA pace.
            hpair = hc[:, 0:H].rearrange("p (o c) -> p o c", o=2)
            hbase = hc[:, 0:H]
            hswap = bass.AP(
                hbase.tensor, hbase.offset + HH,
                [hbase.ap[0], [-HH, 2], [1, HH]],
            )
            cs = hc[:, H:2 * H].rearrange("p (o c) -> p o c", o=2)
            nc.vector.tensor_mul(mA[:], hpair, cs)
            nc.vector.tensor_mul(mB[:], hswap, cs)
            # steady state: sub mostly DVE, add on Pool (balanced just
            # under the DMA pace). For the last tiles the DMA has finished,
            # so everything moves to DVE to drain the backlog fastest.
            sub_eng = nc.gpsimd if (t % 4 == 3 and t < 28) else nc.vector
            add_eng = nc.vector if t >= 30 else nc.gpsimd
            sub_eng.tensor_sub(hr[:, 0:HH], mA[:, 0, :], mA[:, 1, :])
            add_eng.tensor_add(hr[:, HH:H], mB[:, 0, :], mB[:, 1, :])

            # G[:, slice] += hr^T hr[:, slice]; slice = {0:128, 512:640} as
            # one 2-level moving AP. HW: start=True zeroes the WHOLE psum
            # bank, so only the first group touching a bank starts; the
            # co-resident group accumulates onto the zeroed bank.
            hrs = hr[:].rearrange("p (h c) -> p h c", h=2)[:, :, 0:PAIRS]
            for ob in range(8):
                acc = gacc[ob // 2]
                base = (ob % 2) * 256
                nc.tensor.matmul(
                    acc[:, base:base + 256],
                    hr[:, ob * 128:(ob + 1) * 128], hrs,
                    start=(t == 0 and ob % 2 == 0), stop=(t == NT - 1),
                    skip_group_check=True,
                )
            # hT[:, :, t*128:(t+1)*128] = hr^T (PE transpose, bf16 PSUM).
            # The last tile's drains go to Pool so ACT's queue is clear for
            # the G drain the moment the accumulators stop.
            for g in range(2):
                pt = pacc.tile([128, 512], BF16, name="pt", tag="pt", bufs=3)
                for k in range(4):
                    ib = g * 4 + k
                    nc.tensor.transpose(
                        pt[:, k * 128:(k + 1) * 128],
                        hr[:, ib * 128:(ib + 1) * 128],
                        identb[:],
                    )
                dst = hT[:, g * 4:(g + 1) * 4, t * 128:(t + 1) * 128]
                if t >= NT - 1:
                    deferred_ht.append((dst, pt))
                else:
                    nc.scalar.copy(dst, pt[:])

        # W arrives on the same (sync) ring right behind the h stream, in
        # the ob order phase 2 consumes it
        for ob in range(8):
            nc.sync.dma_start(wq_sb[:, ob, :], wq_ap[:, ob, :])

        # ---- phase 2: M[:, slice] = W_q^T G[:, slice] ----
        for gb in range(4):
            if gb < 2:
                nc.vector.tensor_copy(gsl[:, 2 * gb:2 * gb + 2, :], gacc[gb][:])
            else:
                nc.scalar.copy(gsl[:, 2 * gb:2 * gb + 2, :], gacc[gb][:])
        for i, (dst, pt) in enumerate(deferred_ht):
            if i % 2 == 0:
                nc.scalar.copy(dst, pt[:])
            else:
                nc.vector.tensor_copy(dst, pt[:])
        fillers(3, rows=256)
        psm = [
            pacc.tile([128, 512], F32, name=f"psm{i}", tag="acc", bufs=4)
            for i in range(4)
        ]
        for ob in range(8):
            for ib in range(8):
                acc = psm[ib // 2]
                base = (ib % 2) * 256
                nc.tensor.matmul(
                    acc[:, base:base + 256],
                    wq_sb[:, ob, ib * 128:(ib + 1) * 128],
                    gsl[:, ob, :],
                    start=(ob == 0 and ib % 2 == 0), stop=(ob == 7),
                    skip_group_check=True,
                )
            if ob < 2:
                fillers(1, rows=256)
        for gb in range(4):
            if gb % 2 == 0:
                nc.scalar.copy(mq[:, 2 * gb:2 * gb + 2, :], psm[gb][:])
            else:
                nc.vector.tensor_copy(mq[:, 2 * gb:2 * gb + 2, :], psm[gb][:])

        fillers(3, rows=256)

        # ---- phase 3: yT[slice, :] = M[:, slice]^T @ h^T ----
        # transposed orientation: stationary = M column block, moving = hT
        # 512-row chunks — half the matmul count of the y-major form. The
        # host transposes yT back when scattering output columns.
        for lc in range(8):
            for cb in range(2):
                psy = pacc.tile([128, 512], F32, name="psy", tag="acc", bufs=4)
                for ib in range(8):
                    nc.tensor.matmul(
                        psy[:],
                        mq[:, ib, cb * 128:(cb + 1) * 128],
                        hT[:, ib, lc * 512:(lc + 1) * 512],
                        start=(ib == 0), stop=(ib == 7),
                    )
                ysb = stream.tile([128, 512], BF16, name="ysb", tag="yo", bufs=4)
                last = lc == 7 and cb == 1
                if last:
                    # split the final drain across both engines + two DMAs
                    # on separate SEQ rings so the issue chains overlap
                    nc.scalar.copy(ysb[:, 0:256], psy[:, 0:256])
                    nc.vector.tensor_copy(ysb[:, 256:512], psy[:, 256:512])
                    nc.scalar.dma_start(
                        y_ap[cb * 128:(cb + 1) * 128,
                             lc * 512:lc * 512 + 256], ysb[:, 0:256])
                    nc.sync.dma_start(
                        y_ap[cb * 128:(cb + 1) * 128,
                             lc * 512 + 256:(lc + 1) * 512], ysb[:, 256:512])
                else:
                    if (lc * 2 + cb) % 2 == 0:
                        nc.scalar.copy(ysb[:], psy[:])
                    else:
                        nc.vector.tensor_copy(ysb[:], psy[:])
                    nc.scalar.dma_start(
                        y_ap[cb * 128:(cb + 1) * 128, lc * 512:(lc + 1) * 512],
                        ysb[:],
                    )


_NC_CACHE = {}


def _build():
    if "nc" in _NC_CACHE:
        return _NC_CACHE["nc"]
    nc = bacc.Bacc("TRN2", target_bir_lowering=False, debug=False, num_devices=8)
    hcs_d = nc.dram_tensor("hcs", [L, 2 * H], BF16, kind="ExternalInput")
    wq_d = nc.dram_tensor("wq", [H, H], BF16, kind="ExternalInput")
    y_d = nc.dram_tensor("y", [NCS, L], BF16, kind="ExternalOutput")
    with tile.TileContext(nc) as tc:
        _emit_once(nc, tc, hcs_d, wq_d, y_d)
    nc.compile()
    _NC_CACHE["nc"] = nc
    return nc


def _prep_inputs(hidden_states, W_q, cos, sin):
    """Shard/permute on host: per core j, roll the RoPE pair axis by 128*j
    so the core's G/M/y column slice sits at permuted cols {0:128,512:640};
    W_q gets the same permutation on both axes. Pack [h|cos|sin] into one
    row-major bf16 tensor so the stream is one DMA per l-tile."""
    hs = np.asarray(hidden_states, dtype=np.float32)
    wq = np.asarray(W_q, dtype=np.float32)
    c1 = np.asarray(cos, dtype=np.float32)[:, :HH]
    s1 = np.asarray(sin, dtype=np.float32)[:, :HH]

    in_maps = []
    for core in range(8):
        b, j = core // 4, core % 4
        pr = (np.arange(HH) + PAIRS * j) % HH          # pair permutation
        idx = np.concatenate([pr, HH + pr])            # full column perm
        hp = hs[b][:, idx]
        hcs = np.concatenate([hp, c1[:, pr], s1[:, pr]], axis=1)
        wp = wq[np.ix_(idx, idx)]
        in_maps.append({
            "hcs": np.ascontiguousarray(hcs.astype(BF16_NP)),
            "wq": np.ascontiguousarray(wp.astype(BF16_NP)),
        })
    return in_maps


def kernel(hidden_states, W_q, cos, sin):
    in_maps = _prep_inputs(hidden_states, W_q, cos, sin)
    nc = _build()
    res = run_bass_kernel_spmd(nc, in_maps, core_ids=list(range(8)))

    out = np.empty((B, L, H), dtype=np.float32)
    for core, r in enumerate(res.results):
        b, j = core // 4, core % 4
        yt = np.asarray(r["y"]).astype(np.float32)     # [256, L] packed lo|hi
        out[b][:, PAIRS * j:PAIRS * (j + 1)] = yt[0:PAIRS].T
        out[b][:, HH + PAIRS * j:HH + PAIRS * (j + 1)] = yt[PAIRS:NCS].T
    return out
